# revision 4
# baseline (speedup 1.0000x reference)
"""Trainium2 Bass kernel for BufferAttend1d.

reference math (per batch b):
    query = (x @ Wk.T + bk)            [Q, 64]
    keys  = (buffer @ Wk.T + bk)       [K, 64]
    vals  = (buffer @ Wv.T + bv)       [K, 64]
    logits = query @ keys.T / 8        [Q, K]
    logits = where(~mask, logits, -1024)
    probs = softmax(logits, -1)        [Q, K]   (returned)
    read  = probs @ vals               [Q, 64]  (returned)

Strategy: data-parallel over batch (8 cores x 1 batch). On-chip compute is
done entirely in the transposed [k, q] layout so the PV matmul needs no
on-chip transposes:
  - queryT_aug [65, Q]  = (Wk @ x.T + bk)/8 with row 64 = 1.0
  - keysT_aug  [65, K]  = (Wk @ buf.T + bk) with row 64 = -1024*mask
    => logitsT[k, q] = sum_d keysT_aug[d, k] * queryT_aug[d, q]
       (mask folded in as the 65th contraction row)
  - S = exp(logitsT)  (no max subtraction needed: |logits| <= ~4, masked
    entries underflow to exactly 0 like the f32 reference)
  - vals_aug [K, 65] with col 64 = 1.0 => PV matmul gives readT rows 0..63
    and the softmax row-sums in row 64, in one accumulation chain.
  - probsT = S * (1/rowsum) broadcast; readT likewise.
Host transposes x/buffer on the way in and probsT/readT on the way out.
"""

import os
import sys

sys.path.insert(0, "/opt/trn_rl_repo")

from concurrent.futures import ThreadPoolExecutor

import numpy as np
import ml_dtypes

import concourse.bass as bass  # noqa: F401  (bacc subclasses bass)
import concourse.bacc as bacc
import concourse.mybir as mybir
import concourse.tile as tile
from concourse.bass_utils import run_bass_kernel_spmd

B, Q, K, DIN, KD, VD = 8, 4096, 4096, 256, 64, 64
N_CORES = 8
QB = 512                 # q-block (columns per main-loop iteration)
NQB = Q // QB            # 8
KC = 128                 # k-chunk (partitions per PV step)
NKC = K // KC            # 32
VA = VD + 1              # vals augmented with ones column
F32 = mybir.dt.float32
BF16 = mybir.dt.bfloat16
AF = mybir.ActivationFunctionType

# exp groups: one ACT instruction spans EXPG k-chunks worth of logits
EXPG = 2                 # chunks per exp group -> ACT span [128, EXPG*QB]
NEXP = NKC // EXPG

_CACHE: dict = {}


def _build_nc():
    if "nc" in _CACHE:
        return _CACHE["nc"]

    nc = bacc.Bacc("TRN2", target_bir_lowering=False, debug=False,
                   num_devices=N_CORES)

    xT_d = nc.dram_tensor("xT", [DIN, Q], F32, kind="ExternalInput")
    bufT_d = nc.dram_tensor("bufT", [DIN, K], F32, kind="ExternalInput")
    wkT_d = nc.dram_tensor("wkT", [DIN, KD], F32, kind="ExternalInput")
    wvT_d = nc.dram_tensor("wvT", [DIN, VA], F32, kind="ExternalInput")
    bkq_d = nc.dram_tensor("bkq", [KD, 1], F32, kind="ExternalInput")
    bkk_d = nc.dram_tensor("bkk", [KD, 1], F32, kind="ExternalInput")
    bva_d = nc.dram_tensor("bva", [1, VA], F32, kind="ExternalInput")
    mka_d = nc.dram_tensor("maskadd", [1, K], BF16, kind="ExternalInput")

    probsT_d = nc.dram_tensor("probsT", [K, Q], F32, kind="ExternalOutput")
    readT_d = nc.dram_tensor("readT", [VD, Q], F32, kind="ExternalOutput")

    with tile.TileContext(nc) as tc:
        with tc.tile_pool(name="const", bufs=1) as cp:
            # persistent operands
            qTa = cp.tile([KD + 1, Q], BF16)    # queryT augmented (row 64 = 1)
            kTa = cp.tile([KD + 1, K], BF16)    # keysT augmented (row 64 = maskadd)
            vals = cp.tile([128, NKC * VA], BF16)   # 32 chunks of [128, 65]
            ones = cp.tile([1, 128], F32)
            bkq = cp.tile([KD, 1], F32)
            bkk = cp.tile([KD, 1], F32)
            bva = cp.tile([1, VA], F32)
            wk = cp.tile([128, 2 * KD], F32)    # WkT din-chunks side by side
            wv = cp.tile([128, 2 * VA], F32)

            nc.vector.memset(ones[:], 1.0)
            nc.vector.memset(qTa[KD:KD + 1, :], 1.0)
            nc.sync.dma_start(kTa[KD:KD + 1, :], mka_d[:])
            nc.sync.dma_start(bkq[:], bkq_d[:])
            nc.sync.dma_start(bkk[:], bkk_d[:])
            nc.sync.dma_start(bva[:], bva_d[:])
            for c in range(2):
                nc.sync.dma_start(wk[:, c * KD:(c + 1) * KD],
                                  wkT_d[c * 128:(c + 1) * 128, :])
                nc.sync.dma_start(wv[:, c * VA:(c + 1) * VA],
                                  wvT_d[c * 128:(c + 1) * 128, :])

            # ---- setup: projections (transient SBUF + PSUM) ----
            with tc.tile_pool(name="ssb", bufs=1) as ssb, \
                 tc.tile_pool(name="pps", bufs=2, space="PSUM") as pps:
                xt = [ssb.tile([128, Q], F32, tag=f"xt{c}", name=f"xt{c}") for c in range(2)]
                bt = [ssb.tile([128, K], F32, tag=f"bt{c}", name=f"bt{c}") for c in range(2)]
                for c in range(2):
                    nc.sync.dma_start(xt[c][:], xT_d[c * 128:(c + 1) * 128, :])
                    nc.sync.dma_start(bt[c][:], bufT_d[c * 128:(c + 1) * 128, :])

                # queryT and keysT: [64, Q] = WkT.T @ xT (+bias, query scaled 1/8)
                for c0 in range(NQB):
                    sl = slice(c0 * QB, (c0 + 1) * QB)
                    ps = pps.tile([KD, QB], F32, tag="ps")
                    for c in range(2):
                        nc.tensor.matmul(ps[:], wk[:, c * KD:(c + 1) * KD],
                                         xt[c][:, sl],
                                         start=(c == 0), stop=(c == 1))
                    nc.scalar.activation(qTa[0:KD, sl], ps[:], AF.Identity,
                                         bias=bkq[:], scale=0.125)
                    ps2 = pps.tile([KD, QB], F32, tag="ps")
                    for c in range(2):
                        nc.tensor.matmul(ps2[:], wk[:, c * KD:(c + 1) * KD],
                                         bt[c][:, sl],
                                         start=(c == 0), stop=(c == 1))
                    nc.scalar.activation(kTa[0:KD, sl], ps2[:], AF.Identity,
                                         bias=bkk[:], scale=1.0)

                # vals_aug chunks [128, 65] (col 64 = 1.0 via bva trick)
                for j in range(NKC):
                    vp = pps.tile([128, VA], F32, tag="vp")
                    ksl = slice(j * KC, (j + 1) * KC)
                    nc.tensor.matmul(vp[:], bt[0][:, ksl], wv[:, 0:VA],
                                     start=True, stop=False)
                    nc.tensor.matmul(vp[:], bt[1][:, ksl], wv[:, VA:2 * VA],
                                     start=False, stop=False)
                    nc.tensor.matmul(vp[:], ones[:], bva[:],
                                     start=False, stop=True)
                    nc.vector.tensor_copy(vals[:, j * VA:(j + 1) * VA], vp[:])

            # ---- main loop ----
            with tc.tile_pool(name="lg", bufs=3, space="PSUM") as lgp, \
                 tc.tile_pool(name="sp", bufs=2, space="PSUM") as spp, \
                 tc.tile_pool(name="spool", bufs=2) as s_pool, \
                 tc.tile_pool(name="wp", bufs=3) as wp:
                for qb in range(NQB):
                    qsl = slice(qb * QB, (qb + 1) * QB)
                    S = s_pool.tile([128, NKC * QB], BF16, tag="S")
                    for g in range(NEXP):
                        lg = lgp.tile([128, EXPG * QB], F32, tag="lg")
                        for h in range(EXPG):
                            kc = EXPG * g + h
                            nc.tensor.matmul(
                                lg[:, h * QB:(h + 1) * QB],
                                kTa[:, kc * KC:(kc + 1) * KC],
                                qTa[:, qsl], start=True, stop=True)
                        nc.scalar.activation(
                            S[:, g * EXPG * QB:(g + 1) * EXPG * QB],
                            lg[:], AF.Exp)

                    # PV with fused row-sums (rt row 64)
                    rt = spp.tile([128, QB], F32, tag="sp")
                    for j in range(NKC):
                        nc.tensor.matmul(rt[0:VA, :],
                                         vals[:, j * VA:(j + 1) * VA],
                                         S[:, j * QB:(j + 1) * QB],
                                         start=(j == 0), stop=(j == NKC - 1))

                    recip = wp.tile([1, QB], F32, tag="recip")
                    nc.vector.reciprocal(recip[:], rt[VD:VD + 1, :])
                    rb = spp.tile([128, QB], F32, tag="sp")
                    nc.tensor.matmul(rb[:], ones[:], recip[:],
                                     start=True, stop=True)
                    recipB = wp.tile([128, QB], BF16, tag="recipB")
                    nc.vector.tensor_copy(recipB[:], rb[:])

                    readT_sb = wp.tile([VA, QB], F32, tag="readT")
                    nc.vector.tensor_mul(readT_sb[:], rt[0:VA, :],
                                         recipB[0:VA, :])
                    nc.sync.dma_start(readT_d[:, qsl], readT_sb[0:VD, :])

                    # normalize probsT and stream out (4 k-chunks per DMA)
                    for j0 in range(0, NKC, 4):
                        po = wp.tile([128, 4 * QB], F32, tag="po")
                        for jj in range(4):
                            j = j0 + jj
                            nc.vector.tensor_mul(
                                po[:, jj * QB:(jj + 1) * QB],
                                S[:, j * QB:(j + 1) * QB], recipB[:])
                        # SBUF side stays 2D [p, (j c)]; DRAM side iterates
                        # (p, j, c) to match: element (p, j*QB+c) -> row
                        # j*KC+p, col c.
                        out_ap = probsT_d[j0 * KC:(j0 + 4) * KC, qsl] \
                            .rearrange("(j p) c -> p j c", p=KC)
                        nc.sync.dma_start(out_ap, po[:])

    nc.compile()
    _CACHE["nc"] = nc
    return nc


def _prepare_in_maps(x, buffer, mask, Wk, bk, Wv, bv):
    bf16 = ml_dtypes.bfloat16
    f32 = np.float32
    wkT = np.ascontiguousarray(Wk.T.astype(f32))                  # [256, 64]
    wvT = np.ascontiguousarray(
        np.concatenate([Wv.T, np.zeros((DIN, 1), f32)], axis=1))  # [256, 65]
    # scale 1/sqrt(KEY_DIM)=1/8 is folded into the query projection only
    bkq = np.ascontiguousarray((bk.astype(f32) / f32(8.0)).reshape(KD, 1))
    bkk = np.ascontiguousarray(bk.astype(f32).reshape(KD, 1))
    bva = np.ascontiguousarray(
        np.concatenate([bv.astype(f32), np.ones(1, f32)]).reshape(1, VA))
    mka = (f32(-1024.0) * mask.astype(f32)).astype(bf16)          # [B, K]

    xT = np.ascontiguousarray(x.transpose(0, 2, 1).astype(f32))   # [B, 256, Q]
    bufT = np.ascontiguousarray(buffer.transpose(0, 2, 1).astype(f32))

    in_maps = []
    for b in range(B):
        in_maps.append({
            "xT": xT[b], "bufT": bufT[b],
            "wkT": wkT, "wvT": wvT,
            "bkq": bkq, "bkk": bkk, "bva": bva,
            "maskadd": mka[b].reshape(1, K),
        })
    return in_maps


def kernel(x, buffer, mask, Wk, bk, Wv, bv):
    x = np.asarray(x); buffer = np.asarray(buffer); mask = np.asarray(mask)
    Wk = np.asarray(Wk); bk = np.asarray(bk)
    Wv = np.asarray(Wv); bv = np.asarray(bv)

    nc = _build_nc()
    in_maps = _prepare_in_maps(x, buffer, mask, Wk, bk, Wv, bv)
    res = run_bass_kernel_spmd(nc, in_maps, list(range(N_CORES)))

    probs = np.empty((B, Q, K), np.float32)
    read = np.empty((B, Q, VD), np.float32)

    def _assemble(b):
        probs[b] = res.results[b]["probsT"].T
        read[b] = res.results[b]["readT"].T

    with ThreadPoolExecutor(max_workers=8) as ex:
        list(ex.map(_assemble, range(B)))
    return probs, read


if __name__ == "__main__":
    # quick self-run with random data of the right shapes
    rng = np.random.default_rng(0)
    ins = {
        "x": rng.standard_normal((B, Q, DIN), dtype=np.float32),
        "buffer": rng.standard_normal((B, K, DIN), dtype=np.float32),
        "mask": rng.integers(0, 2, (B, K)).astype(bool),
        "Wk": rng.uniform(-0.06, 0.06, (KD, DIN)).astype(np.float32),
        "bk": rng.uniform(-0.06, 0.06, KD).astype(np.float32),
        "Wv": rng.uniform(-0.06, 0.06, (VD, DIN)).astype(np.float32),
        "bv": rng.uniform(-0.06, 0.06, VD).astype(np.float32),
    }
    p, r = kernel(**ins)
    print("probs", p.shape, p.dtype, "read", r.shape, r.dtype)


# revision 5
# speedup vs baseline: 2514.9362x; 2514.9362x over previous
"""Trainium2 Bass kernel for BufferAttend1d.

reference math (per batch b):
    query = (x @ Wk.T + bk)            [Q, 64]
    keys  = (buffer @ Wk.T + bk)       [K, 64]
    vals  = (buffer @ Wv.T + bv)       [K, 64]
    logits = query @ keys.T / 8        [Q, K]
    logits = where(~mask, logits, -1024)
    probs = softmax(logits, -1)        [Q, K]   (returned)
    read  = probs @ vals               [Q, 64]  (returned)

Strategy: data-parallel over batch (8 cores x 1 batch). On-chip compute is
done entirely in the transposed [k, q] layout so the PV matmul needs no
on-chip transposes:
  - queryT [64, Q] = (Wk @ x.T + bk)/8, keysT [64, K] = Wk @ buf.T + bk
    => logitsT[k, q] = sum_d keysT[d, k] * queryT[d, q]
  - masking: in [k, q] layout the mask is per-partition, so it folds into
    the exp's per-partition bias: S = exp(logitsT + maskadd_k). No max
    subtraction needed (|logits| <= ~4); masked entries underflow to
    exactly 0 like the f32 reference.
  - vals_aug [K, 65] with col 64 = 1.0 => PV matmul gives readT rows 0..63
    and the softmax row-sums in row 64, in one accumulation chain.
  - probsT = S * (1/rowsum) broadcast; readT likewise.
Host transposes x/buffer on the way in and probsT/readT on the way out.
"""

import os
import sys

sys.path.insert(0, "/opt/trn_rl_repo")

from concurrent.futures import ThreadPoolExecutor

import numpy as np
import ml_dtypes

import concourse.bass as bass  # noqa: F401  (bacc subclasses bass)
import concourse.bacc as bacc
import concourse.mybir as mybir
import concourse.tile as tile
from concourse.bass_utils import run_bass_kernel_spmd

B, Q, K, DIN, KD, VD = 8, 4096, 4096, 256, 64, 64
N_CORES = 8
QS = 1024                # q-superblock (columns per main-loop iteration)
NQS = Q // QS            # 4
HB = 512                 # psum half-bank width (one matmul's max free dim)
KC = 128                 # k-chunk (partitions per PV step)
NKC = K // KC            # 32
VA = VD + 1              # vals augmented with ones column
F32 = mybir.dt.float32
BF16 = mybir.dt.bfloat16
AF = mybir.ActivationFunctionType

_CACHE: dict = {}


def _build_nc(repeat: int = 1):
    key = ("nc", repeat)
    if key in _CACHE:
        return _CACHE[key]

    nc = bacc.Bacc("TRN2", target_bir_lowering=False, debug=False,
                   num_devices=N_CORES)

    xT_d = nc.dram_tensor("xT", [DIN, Q], F32, kind="ExternalInput")
    bufT_d = nc.dram_tensor("bufT", [DIN, K], F32, kind="ExternalInput")
    wkT_d = nc.dram_tensor("wkT", [DIN, KD], F32, kind="ExternalInput")
    wvT_d = nc.dram_tensor("wvT", [DIN, VA], F32, kind="ExternalInput")
    bkq_d = nc.dram_tensor("bkq", [KD, 1], F32, kind="ExternalInput")
    bkk_d = nc.dram_tensor("bkk", [KD, 1], F32, kind="ExternalInput")
    bva_d = nc.dram_tensor("bva", [1, VA], F32, kind="ExternalInput")
    mka_d = nc.dram_tensor("maskadd", [KC, NKC], F32, kind="ExternalInput")

    probsT_d = nc.dram_tensor("probsT", [K, Q], F32, kind="ExternalOutput")
    readT_d = nc.dram_tensor("readT", [VD, Q], F32, kind="ExternalOutput")

    with tile.TileContext(nc) as tc:
        with tc.tile_pool(name="const", bufs=1) as cp:
            # persistent operands
            qTa = cp.tile([KD, Q], BF16)        # queryT (scaled 1/8)
            kTa = cp.tile([KD, K], BF16)        # keysT
            vals = cp.tile([128, NKC * VA], BF16)   # 32 chunks of [128, 65]
            mka = cp.tile([KC, NKC], F32)       # per-partition mask bias
            ones = cp.tile([1, 128], F32)
            bkq = cp.tile([KD, 1], F32)
            bkk = cp.tile([KD, 1], F32)
            bva = cp.tile([1, VA], F32)
            wk = cp.tile([128, 2 * KD], F32)    # WkT din-chunks side by side
            wv = cp.tile([128, 2 * VA], F32)

            nc.vector.memset(ones[:], 1.0)
            nc.sync.dma_start(mka[:], mka_d[:])
            nc.sync.dma_start(bkq[:], bkq_d[:])
            nc.sync.dma_start(bkk[:], bkk_d[:])
            nc.sync.dma_start(bva[:], bva_d[:])
            for c in range(2):
                nc.sync.dma_start(wk[:, c * KD:(c + 1) * KD],
                                  wkT_d[c * 128:(c + 1) * 128, :])
                nc.sync.dma_start(wv[:, c * VA:(c + 1) * VA],
                                  wvT_d[c * 128:(c + 1) * 128, :])

            # ---- setup: projections (transient SBUF + PSUM) ----
            with tc.tile_pool(name="ssb", bufs=1) as ssb, \
                 tc.tile_pool(name="pps", bufs=2, space="PSUM") as pps:
                xt = [ssb.tile([128, Q], F32, tag=f"xt{c}", name=f"xt{c}")
                      for c in range(2)]
                bt = [ssb.tile([128, K], F32, tag=f"bt{c}", name=f"bt{c}")
                      for c in range(2)]
                for c in range(2):
                    nc.sync.dma_start(xt[c][:], xT_d[c * 128:(c + 1) * 128, :])
                    nc.sync.dma_start(bt[c][:], bufT_d[c * 128:(c + 1) * 128, :])

                # queryT and keysT: [64, Q] = WkT.T @ xT (+bias, query /8)
                for c0 in range(Q // HB):
                    sl = slice(c0 * HB, (c0 + 1) * HB)
                    ps = pps.tile([KD, HB], F32, tag="ps")
                    for c in range(2):
                        nc.tensor.matmul(ps[:], wk[:, c * KD:(c + 1) * KD],
                                         xt[c][:, sl],
                                         start=(c == 0), stop=(c == 1))
                    nc.scalar.activation(qTa[:, sl], ps[:], AF.Identity,
                                         bias=bkq[:], scale=0.125)
                    ps2 = pps.tile([KD, HB], F32, tag="ps")
                    for c in range(2):
                        nc.tensor.matmul(ps2[:], wk[:, c * KD:(c + 1) * KD],
                                         bt[c][:, sl],
                                         start=(c == 0), stop=(c == 1))
                    nc.scalar.activation(kTa[:, sl], ps2[:], AF.Identity,
                                         bias=bkk[:], scale=1.0)

                # vals_aug chunks [128, 65] (col 64 = 1.0 via bva trick)
                for j in range(NKC):
                    vp = pps.tile([128, VA], F32, tag="vp")
                    ksl = slice(j * KC, (j + 1) * KC)
                    nc.tensor.matmul(vp[:], bt[0][:, ksl], wv[:, 0:VA],
                                     start=True, stop=False)
                    nc.tensor.matmul(vp[:], bt[1][:, ksl], wv[:, VA:2 * VA],
                                     start=False, stop=False)
                    nc.tensor.matmul(vp[:], ones[:], bva[:],
                                     start=False, stop=True)
                    nc.vector.tensor_copy(vals[:, j * VA:(j + 1) * VA], vp[:])

            # ---- main loop ----
            with tc.tile_pool(name="lg", bufs=2, space="PSUM") as lgp, \
                 tc.tile_pool(name="sp", bufs=1, space="PSUM") as spp, \
                 tc.tile_pool(name="spool", bufs=2) as s_pool, \
                 tc.tile_pool(name="wp", bufs=3) as wp:

                def main_body():
                    for qs in range(NQS):
                        qsl = slice(qs * QS, (qs + 1) * QS)
                        S = s_pool.tile([128, NKC * QS], BF16, tag="S",
                                        name="S")
                        for j in range(NKC):
                            lg = lgp.tile([128, QS], F32, tag="lg", name="lg")
                            for h in range(2):
                                nc.tensor.matmul(
                                    lg[:, h * HB:(h + 1) * HB],
                                    kTa[:, j * KC:(j + 1) * KC],
                                    qTa[:, qs * QS + h * HB:
                                        qs * QS + (h + 1) * HB],
                                    start=True, stop=True)
                            # exp with mask folded in as per-partition bias
                            nc.scalar.activation(S[:, j * QS:(j + 1) * QS],
                                                 lg[:], AF.Exp,
                                                 bias=mka[:, j:j + 1],
                                                 scale=1.0)

                        # PV with fused row-sums (rt row 64)
                        rt = spp.tile([128, QS], F32, tag="rt", name="rt")
                        for j in range(NKC):
                            for h in range(2):
                                nc.tensor.matmul(
                                    rt[0:VA, h * HB:(h + 1) * HB],
                                    vals[:, j * VA:(j + 1) * VA],
                                    S[:, j * QS + h * HB:j * QS + (h + 1) * HB],
                                    start=(j == 0), stop=(j == NKC - 1))

                        recip = wp.tile([1, QS], F32, tag="recip", name="recip")
                        nc.vector.reciprocal(recip[:], rt[VD:VD + 1, :])
                        rb = spp.tile([128, QS], F32, tag="rb", name="rb")
                        for h in range(2):
                            nc.tensor.matmul(rb[:, h * HB:(h + 1) * HB],
                                             ones[:],
                                             recip[:, h * HB:(h + 1) * HB],
                                             start=True, stop=True)
                        recipB = wp.tile([128, QS], BF16, tag="recipB",
                                         name="recipB")
                        nc.vector.tensor_copy(recipB[:], rb[:])

                        readT_sb = wp.tile([VA, QS], F32, tag="readT",
                                           name="readT")
                        nc.vector.tensor_mul(readT_sb[:], rt[0:VA, :],
                                             recipB[0:VA, :])
                        nc.sync.dma_start(readT_d[:, qsl], readT_sb[0:VD, :])

                        # normalize probsT and stream out (2 k-chunks per DMA)
                        for j0 in range(0, NKC, 2):
                            po = wp.tile([128, 2 * QS], F32, tag="po",
                                         name="po")
                            for jj in range(2):
                                j = j0 + jj
                                nc.vector.tensor_mul(
                                    po[:, jj * QS:(jj + 1) * QS],
                                    S[:, j * QS:(j + 1) * QS], recipB[:])
                            # SBUF side stays 2D [p, (j c)]; DRAM side
                            # iterates (p, j, c): elem (p, j*QS+c) -> row
                            # j0*KC + j*KC + p, col qs*QS + c.
                            out_ap = probsT_d[j0 * KC:(j0 + 2) * KC, qsl] \
                                .rearrange("(j p) c -> p j c", p=KC)
                            nc.sync.dma_start(out_ap, po[:])

                if repeat == 1:
                    main_body()
                else:
                    with tc.For_i(0, repeat, 1):
                        main_body()

    nc.compile()
    _CACHE[key] = nc
    return nc


def _prepare_in_maps(x, buffer, mask, Wk, bk, Wv, bv):
    f32 = np.float32
    wkT = np.ascontiguousarray(Wk.T.astype(f32))                  # [256, 64]
    wvT = np.ascontiguousarray(
        np.concatenate([Wv.T, np.zeros((DIN, 1), f32)], axis=1))  # [256, 65]
    # scale 1/sqrt(KEY_DIM)=1/8 is folded into the query projection only
    bkq = np.ascontiguousarray((bk.astype(f32) / f32(8.0)).reshape(KD, 1))
    bkk = np.ascontiguousarray(bk.astype(f32).reshape(KD, 1))
    bva = np.ascontiguousarray(
        np.concatenate([bv.astype(f32), np.ones(1, f32)]).reshape(1, VA))
    # per-partition mask bias: mka[p, j] = -1024 * mask[128j + p]
    mka = (f32(-1024.0) * mask.astype(f32)).reshape(B, NKC, KC) \
        .transpose(0, 2, 1)                                       # [B, 128, 32]
    mka = np.ascontiguousarray(mka)

    xT = np.ascontiguousarray(x.transpose(0, 2, 1).astype(f32))   # [B, 256, Q]
    bufT = np.ascontiguousarray(buffer.transpose(0, 2, 1).astype(f32))

    in_maps = []
    for b in range(B):
        in_maps.append({
            "xT": xT[b], "bufT": bufT[b],
            "wkT": wkT, "wvT": wvT,
            "bkq": bkq, "bkk": bkk, "bva": bva,
            "maskadd": mka[b],
        })
    return in_maps


def kernel(x, buffer, mask, Wk, bk, Wv, bv):
    x = np.asarray(x); buffer = np.asarray(buffer); mask = np.asarray(mask)
    Wk = np.asarray(Wk); bk = np.asarray(bk)
    Wv = np.asarray(Wv); bv = np.asarray(bv)

    nc = _build_nc()
    in_maps = _prepare_in_maps(x, buffer, mask, Wk, bk, Wv, bv)
    res = run_bass_kernel_spmd(nc, in_maps, list(range(N_CORES)))

    probs = np.empty((B, Q, K), np.float32)
    read = np.empty((B, Q, VD), np.float32)

    def _assemble(b):
        probs[b] = res.results[b]["probsT"].T
        read[b] = res.results[b]["readT"].T

    with ThreadPoolExecutor(max_workers=8) as ex:
        list(ex.map(_assemble, range(B)))
    return probs, read


if __name__ == "__main__":
    rng = np.random.default_rng(0)
    ins = {
        "x": rng.standard_normal((B, Q, DIN), dtype=np.float32),
        "buffer": rng.standard_normal((B, K, DIN), dtype=np.float32),
        "mask": rng.integers(0, 2, (B, K)).astype(bool),
        "Wk": rng.uniform(-0.06, 0.06, (KD, DIN)).astype(np.float32),
        "bk": rng.uniform(-0.06, 0.06, KD).astype(np.float32),
        "Wv": rng.uniform(-0.06, 0.06, (VD, DIN)).astype(np.float32),
        "bv": rng.uniform(-0.06, 0.06, VD).astype(np.float32),
    }
    p, r = kernel(**ins)
    print("probs", p.shape, p.dtype, "read", r.shape, r.dtype)
